# revision 4
# baseline (speedup 1.0000x reference)
"""Trainium2 Bass kernel for 12-head attention (B=8, N=1024, D=768). v2.

Data-parallel over batch: each core does one [1024, 768] batch element.

Schedule (single pass, PE-dense):
  startup: x^T via f32r PE transposes; V = x @ w_v (bf16 + ones cols);
           Q^T/K^T for head pair 0.
  pairs 0..5: S^T -> exp (ACT) -> O~^T accumulation, with the NEXT pair's
           Q^T/K^T matmuls interleaved as background PE work so the PE
           never idles while ACT runs exp. Softmax denominators ride the
           ones column (row 64 of each O~ psum tile).
  norm(pr) lagged one pair: denominators + reciprocals live in one
           [128,1024] tile per pair at partitions {0,32,64,96} (32-aligned
           so engine copies can shift partitions; no SBUF-SBUF DMAs).
  proj:    out = O^T-as-lhsT @ w_proj, kt-outer / st-inner for 8x weight
           reuse, output DMA per [128,512] chunk overlaps the nb=1 pass.

ACT = exp + denominator row extraction only; DVE owns all PSUM evictions,
reciprocals and norm multiplies; weights stream on the scalar HWDGE queue,
x / out on SP.
"""

import numpy as np

import concourse.bass as bass
from concourse import bacc
import concourse.mybir as mybir
import concourse.tile as tile
from concourse.masks import make_identity

F32 = mybir.dt.float32
F32R = mybir.dt.float32r
BF16 = mybir.dt.bfloat16
AF = mybir.ActivationFunctionType

N = 1024   # sequence length
D = 768    # model dim
H = 12     # heads
HD = 64    # head dim
NT = N // 128   # 8 seq tiles
DT = D // 128   # 6 dim tiles
SCALE = HD ** -0.5  # 0.125
VPW = H * (HD + 1)  # 780: per-head 64 V cols + ones col
NPAIR = H // 2


def _r(ap):
    return ap.bitcast(F32R)


def _f(ap):
    return ap.bitcast(F32)


def build_module(with_bias: bool, loop_iters: int = 0) -> bass.Bass:
    nc = bacc.Bacc("TRN2", target_bir_lowering=False, debug=False)

    x_d = nc.dram_tensor("x", [N, D], F32, kind="ExternalInput")
    wqkv_d = nc.dram_tensor("w_qkv", [D, 3 * D], F32, kind="ExternalInput")
    bqkv_d = nc.dram_tensor("b_qkv", [1, 3 * D], F32, kind="ExternalInput")
    wp_d = nc.dram_tensor("w_proj", [D, D], F32, kind="ExternalInput")
    bp_d = nc.dram_tensor("b_proj", [1, D], F32, kind="ExternalInput")
    out_d = nc.dram_tensor("out", [N, D], F32, kind="ExternalOutput")

    with tile.TileContext(nc) as tc:
        if loop_iters:
            with tc.For_i(0, loop_iters, 1, hint_engines=(mybir.EngineType.PE,)):
                _emit(nc, tc, x_d, wqkv_d, bqkv_d, wp_d, bp_d, out_d, with_bias)
        else:
            _emit(nc, tc, x_d, wqkv_d, bqkv_d, wp_d, bp_d, out_d, with_bias)
    nc.compile()
    return nc


def _emit(nc, tc, x_d, wqkv_d, bqkv_d, wp_d, bp_d, out_d, with_bias):
    # ---- persistent tensors ----
    top = tc.alloc_tile_pool(name="top", bufs=1)
    identity = top.tile([128, 128], F32, name="identity")
    make_identity(nc, identity)
    ones = top.tile([128, 512], F32, name="ones")
    nc.gpsimd.memset(ones, 1.0)
    ones_r = top.tile([1, HD], F32R, name="ones_r")
    nc.vector.tensor_copy(ones_r, ones[0:1, 0:HD])

    qt_sb = top.tile([128, DT, N], F32R, name="qt_sb")    # Q^T [768, 1024]
    kt_sb = top.tile([128, DT, N], F32R, name="kt_sb")    # K^T [768, 1024]
    vp_sb = top.tile([128, NT, VPW], BF16, name="vp_sb")  # V' bf16
    ot_sb = top.tile([128, DT, N], F32R, name="ot_sb")    # O^T [768, 1024]
    xt_pool = tc.alloc_tile_pool(name="xtp", bufs=1)
    xt_sb = xt_pool.tile([128, DT, N], F32R, name="xt_sb")  # x^T [768, 1024]

    if with_bias:
        bq_row = top.tile([1, 3 * D], F32, name="bq_row")
        bp_row = top.tile([1, D], F32, name="bp_row")
        nc.scalar.dma_start(bq_row, bqkv_d.ap())
        nc.scalar.dma_start(bp_row, bp_d.ap())
    else:
        bq_row = bp_row = None

    # ---- weight pools & DMAs (scalar HWDGE queue; x and out ride SP) ----
    # wqk split: pool a = pairs 0-2, pool b = pairs 3-5; a is released
    # mid-kernel to make room for w_proj.
    wqk_pool = [tc.alloc_tile_pool(name="wqka", bufs=1),
                tc.alloc_tile_pool(name="wqkb", bufs=1)]
    wqk_sb = [p.tile([128, DT, 2 * 384], F32R, name=f"wqk_sb{i}")
              for i, p in enumerate(wqk_pool)]

    def wqk_slice(pr, which):
        """SBUF view [128, DT, 128] of w_{q,k} columns for pair pr."""
        half, mt = divmod(pr, 3)
        c = which * 384 + mt * 128
        return wqk_sb[half][:, :, c:c + 128]

    def dma_wqk(pr, which):
        half, mt = divmod(pr, 3)
        c = which * 384 + mt * 128
        src = wqkv_d.ap()[:, which * D + pr * 128: which * D + (pr + 1) * 128]
        nc.scalar.dma_start(
            wqk_sb[half][:, :, c:c + 128],
            src.rearrange("(ko p) n -> p ko n", p=128).bitcast(F32R))

    # DMA queue plan: a dma_start blocks its issuing engine's queue for the
    # whole transfer in practice, so weights ride the scalar (ACT) queue --
    # which is idle until the first exp at ~28us -- ordered w_v first (V
    # matmuls interleave with the transposes), then pair-0 Q/K; x tiles
    # ride SP.
    late = tc.alloc_tile_pool(name="late", bufs=1)
    wp_sb = late.tile([128, DT, D], F32R, name="wp_sb")

    wv_pool = tc.alloc_tile_pool(name="wv", bufs=1)
    wv_sb = wv_pool.tile([128, DT, D], F32R, name="wv_sb")
    for voff, vw in ((0, 512), (512, 256)):
        nc.scalar.dma_start(
            wv_sb[:, :, voff:voff + vw],
            wqkv_d.ap()[:, 2 * D + voff:2 * D + voff + vw].rearrange(
                "(ko p) n -> p ko n", p=128).bitcast(F32R))
    dma_wqk(0, 0)
    dma_wqk(0, 1)

    for pr in range(1, NPAIR):
        dma_wqk(pr, 0)
        dma_wqk(pr, 1)
    # w_proj queued last on the scalar HWDGE queue: arrives ~20us in,
    # needed only at the proj tail.
    nc.scalar.dma_start(
        wp_sb, wp_d.ap().rearrange("(ko p) n -> p ko n", p=128).bitcast(F32R))

    xs_pool = tc.alloc_tile_pool(name="xs", bufs=3)

    # PSUM pools: qkps (long-lived) 2x1 banks, pt (startup transposes,
    # released first) 2x2 banks.
    qkps = tc.alloc_tile_pool(name="qkps", bufs=2, space="PSUM")
    pt_ps = tc.alloc_tile_pool(name="ptps", bufs=2, space="PSUM")

    vp_view = vp_sb.rearrange("p st (h c) -> p st h c", c=HD + 1)
    nc.gpsimd.memset(vp_view[:, :, :, HD:HD + 1], 1.0)

    exp_warm = top.tile([1, 8], F32, name="exp_warm")
    nc.scalar.activation(exp_warm, ones[0:1, 0:8], AF.Exp, scale=1.0)

    def emit_v(st):
        """V(st) = x[st] @ w_v into vp (bf16, ones cols interleaved).
        nb0 fully before nb1 so only wv[:, :512] gates the start."""
        for nb, (noff, nw) in enumerate(((0, 512), (512, 256))):
            ps = qkps.tile([128, nw], F32, tag="qk", name=f"v_{st}_{nb}")
            for kt_i in range(DT):
                nc.tensor.matmul(
                    ps,
                    xt_sb[:, kt_i, st * 128:(st + 1) * 128],
                    wv_sb[:, kt_i, noff:noff + nw],
                    start=(kt_i == 0),
                    stop=(kt_i == DT - 1 and not with_bias),
                )
            if with_bias:
                nc.tensor.matmul(
                    ps,
                    ones[0:1, 0:128],
                    bq_row[0:1, 2 * D + noff:2 * D + noff + nw],
                    start=False, stop=True,
                )
            h0, hn = noff // HD, nw // HD
            nc.vector.tensor_copy(
                vp_view[:, st, h0:h0 + hn, 0:HD],
                ps.rearrange("p (h c) -> p h c", c=HD),
            )

    # ---- startup: x^T transposes with V matmuls filling the
    # x-DMA arrival gaps; V(st) only needs xt(st) + w_v ----
    for st in range(NT):
        x_t = xs_pool.tile([128, D], F32, tag="xrow", name=f"x_{st}")
        nc.sync.dma_start(x_t, x_d.ap()[st * 128:(st + 1) * 128, :])
        pt = pt_ps.tile([128, D], F32, tag="pt", name=f"pt_{st}")
        for dt_i in range(DT):
            nc.tensor.transpose(
                pt[:, dt_i * 128:(dt_i + 1) * 128],
                x_t[:, dt_i * 128:(dt_i + 1) * 128], identity)
        nc.vector.tensor_copy(
            xt_sb[:, :, st * 128:(st + 1) * 128],
            pt.rearrange("p (d c) -> p d c", c=128))
        emit_v(st)
    xs_pool.release()
    pt_ps.release()

    # ---- QK helper: units for interleaving ----
    qk_state = {}

    def qk_mm(pr, which, kt, qb):
        key = (which, qb)
        if kt == 0:
            qk_state[key] = qkps.tile(
                [128, 512], F32, tag="qk", name=f"qk_{pr}_{which}_{qb}")
        ps = qk_state[key]
        nc.tensor.matmul(
            ps,
            wqk_slice(pr, which)[:, kt, :],
            xt_sb[:, kt, qb * 512:(qb + 1) * 512],
            start=(kt == 0),
            stop=(kt == DT - 1 and not with_bias),
        )

    def qk_bias(pr, which, qb):
        nc.tensor.matmul(
            qk_state[(which, qb)],
            bq_row[0:1, which * D + pr * 128: which * D + (pr + 1) * 128],
            ones[0:1, 0:512],
            start=False, stop=True,
        )

    def qk_evict(pr, which, qb):
        dst = qt_sb if which == 0 else kt_sb
        nc.vector.tensor_copy(
            dst[:, pr, qb * 512:(qb + 1) * 512], qk_state[(which, qb)])

    def make_qk_units(pr):
        units = []
        for which in range(2):
            for kt in range(DT):
                for qb in range(2):
                    units.append(
                        lambda pr=pr, w=which, k=kt, q=qb: qk_mm(pr, w, k, q))
            if with_bias:
                for qb in range(2):
                    units.append(
                        lambda pr=pr, w=which, q=qb: qk_bias(pr, w, q))
            for qb in range(2):
                units.append(
                    lambda pr=pr, w=which, q=qb: qk_evict(pr, w, q))
        return units

    # pair 0 QK after the transpose/V stream
    for u in make_qk_units(0):
        u()
    wv_pool.release()

    # ---- attention pools ----
    s_ps_pool = tc.alloc_tile_pool(name="sps", bufs=2, space="PSUM")
    o_ps_pool = tc.alloc_tile_pool(name="ops", bufs=4, space="PSUM")
    pexp_pool = tc.alloc_tile_pool(name="pexp", bufs=6)
    den_pool = tc.alloc_tile_pool(name="den", bufs=1)
    dens = {}

    def emit_norm(pr):
        """Normalize pair pr's O^T rows: bcast s (den row 0, col half = hh)
        to 64 partitions via K=1 matmuls, reciprocal in PSUM on DVE,
        multiply into O^T on DVE. r_ps borrows the qkps ring (its groups
        close fast, so no ring deadlock mid-pair)."""
        den = dens.pop(pr)
        for qb in range(2):
            r_ps = qkps.tile([128, 512], F32, tag="qk", name=f"r_{pr}_{qb}")
            # f32r matmul can't target PSUM base partition 64, so the hh=1
            # broadcast stays fp32 (4 cyc/row)
            nc.tensor.matmul(
                r_ps[0:64, :],
                ones_r[0:1, :],
                den[0:1, qb * 512:qb * 512 + 512],
                start=True, stop=True,
            )
            nc.tensor.matmul(
                r_ps[64:128, :],
                ones[0:1, 0:HD],
                _f(den[0:1, N + qb * 512:N + qb * 512 + 512]),
                start=True, stop=True,
            )
            nc.vector.reciprocal_approx_fast(out=r_ps, in_=r_ps)
            for hh in range(2):
                po = 64 * hh
                dst = ot_sb[po:po + 64, pr, qb * 512:(qb + 1) * 512]
                nc.vector.tensor_mul(out=dst, in0=_f(dst),
                                     in1=r_ps[po:po + 64, :])

    for pr in range(NPAIR):
        bg = make_qk_units(pr + 1) if pr + 1 < NPAIR else []
        bgi = 0

        def drain(k):
            nonlocal bgi
            while k > 0 and bgi < len(bg):
                bg[bgi]()
                bgi += 1
                k -= 1

        o_ps = {}
        for hh in range(2):
            for qb in range(2):
                o_ps[(hh, qb)] = o_ps_pool.tile(
                    [65, 512], F32, tag="o", name=f"o_{pr}_{hh}_{qb}")

        for kt_i in range(NT):
            pexp = {}
            for hh in range(2):
                po = 64 * hh
                for qb in range(2):
                    s_ps = s_ps_pool.tile([128, 512], F32, tag="s",
                                          name=f"s_{pr}_{kt_i}_{hh}_{qb}")
                    nc.tensor.matmul(
                        s_ps,
                        kt_sb[po:po + 64, pr, kt_i * 128:(kt_i + 1) * 128],
                        qt_sb[po:po + 64, pr, qb * 512:(qb + 1) * 512],
                        start=True, stop=True,
                    )
                    pe = pexp_pool.tile([128, 512], BF16, tag="pexp",
                                        name=f"pe_{pr}_{kt_i}_{hh}_{qb}")
                    nc.scalar.activation(pe, s_ps, AF.Exp, scale=float(SCALE))
                    pexp[(hh, qb)] = pe
                drain(2)

            for hh in range(2):
                h = 2 * pr + hh
                for qb in range(2):
                    nc.tensor.matmul(
                        o_ps[(hh, qb)],
                        vp_sb[:, kt_i, h * (HD + 1):(h + 1) * (HD + 1)],
                        pexp[(hh, qb)],
                        start=(kt_i == 0),
                        stop=(kt_i == NT - 1),
                        skip_group_check=True,
                    )
            drain(2)
            if kt_i == 3 and pr >= 1:
                emit_norm(pr - 1)

        # pair epilogue: evict O~^T, extract denominators, reciprocals
        den = den_pool.tile([1, 2 * N], F32R, tag="den", name=f"den_{pr}")
        dens[pr] = den
        for hh in range(2):
            po = 64 * hh
            for qb in range(2):
                nc.vector.tensor_copy(
                    ot_sb[po:po + 64, pr, qb * 512:(qb + 1) * 512],
                    o_ps[(hh, qb)][0:HD, :])
                dslice = den[0:1, hh * N + qb * 512:hh * N + qb * 512 + 512]
                if pr == NPAIR - 1:
                    # last pair: ACT is idle; halves the epilogue DVE chain
                    nc.scalar.copy(dslice, o_ps[(hh, qb)][HD:HD + 1, :])
                else:
                    nc.vector.tensor_copy(dslice, o_ps[(hh, qb)][HD:HD + 1, :])
        while bgi < len(bg):
            bg[bgi]()
            bgi += 1

    emit_norm(NPAIR - 1)
    o_ps_pool.release()
    s_ps_pool.release()
    qkps.release()

    # ---- proj: kt-outer / st-inner (8x lhsT reuse), DMA out per chunk ----
    proj_ps = tc.alloc_tile_pool(name="projps", bufs=8, space="PSUM")
    fout_pool = tc.alloc_tile_pool(name="fout", bufs=3)
    for noff, nw in ((0, 512), (512, 256)):
        tiles = [proj_ps.tile([128, nw], F32, tag="pj", name=f"pj_{noff}_{st}")
                 for st in range(NT)]
        for kt_i in range(DT):
            for st in range(NT):
                nc.tensor.matmul(
                    tiles[st],
                    ot_sb[:, kt_i, st * 128:(st + 1) * 128],
                    wp_sb[:, kt_i, noff:noff + nw],
                    start=(kt_i == 0),
                    stop=(kt_i == DT - 1 and not with_bias),
                )
                if kt_i == DT - 1:
                    # evict + store as soon as each st group closes so the
                    # DVE/DMA drain overlaps the remaining matmuls
                    if with_bias:
                        nc.tensor.matmul(
                            tiles[st],
                            ones[0:1, 0:128],
                            bp_row[0:1, noff:noff + nw],
                            start=False, stop=True,
                        )
                    fo = fout_pool.tile([128, nw], F32, tag="fo",
                                        name=f"fo_{noff}_{st}")
                    nc.vector.tensor_copy(fo, tiles[st])
                    nc.sync.dma_start(
                        out_d.ap()[st * 128:(st + 1) * 128, noff:noff + nw],
                        fo)

    fout_pool.release()
    proj_ps.release()
    den_pool.release()
    pexp_pool.release()
    late.release()
    wqk_pool[1].release()
    wqk_pool[0].release()
    xt_pool.release()
    top.release()


_module_cache: dict = {}


def get_module(with_bias: bool) -> bass.Bass:
    if with_bias not in _module_cache:
        _module_cache[with_bias] = build_module(with_bias)
    return _module_cache[with_bias]


def kernel(x, w_qkv, b_qkv, w_proj, b_proj):
    from concourse.bass_utils import run_bass_kernel_spmd

    x = np.ascontiguousarray(np.asarray(x, dtype=np.float32))
    w_qkv = np.ascontiguousarray(np.asarray(w_qkv, dtype=np.float32))
    b_qkv = np.ascontiguousarray(
        np.asarray(b_qkv, dtype=np.float32)).reshape(1, 3 * D)
    w_proj = np.ascontiguousarray(np.asarray(w_proj, dtype=np.float32))
    b_proj = np.ascontiguousarray(
        np.asarray(b_proj, dtype=np.float32)).reshape(1, D)

    B = x.shape[0]
    assert x.shape == (B, N, D) and B == 8, x.shape

    with_bias = bool(np.any(b_qkv) or np.any(b_proj))
    nc = get_module(with_bias)

    in_maps = [
        {
            "x": np.ascontiguousarray(x[b]),
            "w_qkv": w_qkv,
            "b_qkv": b_qkv,
            "w_proj": w_proj,
            "b_proj": b_proj,
        }
        for b in range(B)
    ]
    res = run_bass_kernel_spmd(nc, in_maps, core_ids=list(range(B)))
    kernel.last_results = res
    return np.stack([res.results[b]["out"] for b in range(B)], axis=0)


# revision 5
# speedup vs baseline: 1.2459x; 1.2459x over previous
"""Trainium2 Bass kernel for 12-head attention (B=8, N=1024, D=768).

Sharding: data-parallel over batch — each of the 8 NeuronCores processes one
batch element [1024, 768] end-to-end; weights are replicated. No collectives.

Per-core algorithm (matmuls in float32r = FP22, full PE rate at N>=256):
  1. x^T via PE transposes, interleaved per seq-tile with the V matmuls.
  2. Q^T = w_q-as-lhsT @ x^T -> [768, 1024]; K^T likewise.
     V = x @ w_v -> [1024, 768], stored bf16, interleaved per head with a
     ones column: V'[:, 65h:65h+64] = V_h, V'[:, 65h+64] = 1.
  3. Per head pair (heads 2t/2t+1 live on partitions 0-63/64-127, so their
     K=64 S-matmuls share the PE via row-group tiling):
     S^T[kt] = K_h tile-as-lhsT @ Q_h^T -> PSUM [128, 1024]
     P~^T[kt] = exp(S^T * 1/8)            (ScalarE, PSUM->SBUF bf16)
     O~'[h,qb] += V'_h[kt]-as-lhsT @ P~^T[kt] -> PSUM [65, 512]
     rows 0-63 = unnormalized O^T head rows, row 64 = softmax denominators.
  4. Denominators -> 1/s via DVE reciprocal_approx_fast ([12, 1024] batched),
     broadcast to 64 partitions via K=1 ones matmul, multiplied into O^T.
  5. out = O^T-as-lhsT @ w_proj -> [1024, 768] -> HBM.

Biases enter as K=1 matmuls appended to each accumulation group (skipped
when the host sees all-zero biases, which is what the reference generates).
"""

import os
import numpy as np

import concourse.bass as bass
from concourse import bacc
import concourse.mybir as mybir
import concourse.tile as tile
from concourse.masks import make_identity

F32 = mybir.dt.float32
F32R = mybir.dt.float32r
BF16 = mybir.dt.bfloat16
AF = mybir.ActivationFunctionType

N = 1024   # sequence length
D = 768    # model dim
H = 12     # heads
HD = 64    # head dim
NT = N // 128   # 8 seq tiles
DT = D // 128   # 6 dim tiles
SCALE = HD ** -0.5  # 0.125
VPW = H * (HD + 1)  # 780: per-head 64 V cols + ones col


def _r(ap):
    """Reinterpret an fp32 AP as float32r for full-rate PE matmuls."""
    return ap.bitcast(F32R)


def build_module(with_bias: bool, loop_iters: int = 0) -> bass.Bass:
    nc = bacc.Bacc("TRN2", target_bir_lowering=False, debug=False)

    x_d = nc.dram_tensor("x", [N, D], F32, kind="ExternalInput")
    wqkv_d = nc.dram_tensor("w_qkv", [D, 3 * D], F32, kind="ExternalInput")
    bqkv_d = nc.dram_tensor("b_qkv", [1, 3 * D], F32, kind="ExternalInput")
    wp_d = nc.dram_tensor("w_proj", [D, D], F32, kind="ExternalInput")
    bp_d = nc.dram_tensor("b_proj", [1, D], F32, kind="ExternalInput")
    out_d = nc.dram_tensor("out", [N, D], F32, kind="ExternalOutput")

    with tile.TileContext(nc) as tc:
        if loop_iters:
            with tc.For_i(0, loop_iters, 1, hint_engines=(mybir.EngineType.PE,)):
                _emit(nc, tc, x_d, wqkv_d, bqkv_d, wp_d, bp_d, out_d,
                      with_bias)
        else:
            _emit(nc, tc, x_d, wqkv_d, bqkv_d, wp_d, bp_d, out_d, with_bias)
    nc.compile()
    return nc


def _emit(nc, tc, x_d, wqkv_d, bqkv_d, wp_d, bp_d, out_d, with_bias):
    # ---- persistent pools / tensors; big weight DMAs issued first ----
    top = tc.alloc_tile_pool(name="top", bufs=1)
    identity = top.tile([128, 128], F32, name="identity")
    make_identity(nc, identity)
    ones = top.tile([1, 512], F32, name="ones")
    nc.gpsimd.memset(ones, 1.0)

    qt_sb = top.tile([128, DT, N], F32R, name="qt_sb")    # Q^T [768, 1024]
    kt_sb = top.tile([128, DT, N], F32R, name="kt_sb")    # K^T [768, 1024]
    vp_sb = top.tile([128, NT, VPW], BF16, name="vp_sb")  # V' bf16
    ot_sb = top.tile([128, DT, N], F32R, name="ot_sb")    # O^T [768, 1024]

    if with_bias:
        bq_row = top.tile([1, 3 * D], F32, name="bq_row")
        bp_row = top.tile([1, D], F32, name="bp_row")
        nc.scalar.dma_start(bq_row, bqkv_d.ap())
        nc.scalar.dma_start(bp_row, bp_d.ap())
    else:
        bq_row = bp_row = None

    # Weights go through ScalarE's HWDGE queue so the x-tile loads on SP's
    # queue aren't stuck behind 7 MB of weight traffic.
    xt_pool = tc.alloc_tile_pool(name="xtp", bufs=1)
    xt_sb = xt_pool.tile([128, DT, N], F32R, name="xt_sb")  # x^T [768, 1024]

    wv_pool0 = tc.alloc_tile_pool(name="wv", bufs=1)
    wv_sb = wv_pool0.tile([128, DT, D], F32R, name="wv_sb")
    for voff, vw in ((0, 512), (512, 256)):
        nc.scalar.dma_start(
            wv_sb[:, :, voff:voff + vw],
            wqkv_d.ap()[:, 2 * D + voff:2 * D + voff + vw].rearrange(
                "(ko p) n -> p ko n", p=128).bitcast(F32R))

    wqk_pool = tc.alloc_tile_pool(name="wqk", bufs=1)
    wqk_sb = wqk_pool.tile([128, DT, 2 * D], F32R, name="wqk_sb")
    for wh in range(2):  # w_q then w_k, so the Q matmuls can start sooner
        nc.scalar.dma_start(
            wqk_sb[:, :, wh * D:(wh + 1) * D],
            wqkv_d.ap()[:, wh * D:(wh + 1) * D].rearrange(
                "(ko p) n -> p ko n", p=128).bitcast(F32R))

    xs_pool = tc.alloc_tile_pool(name="xs", bufs=4)

    psA = tc.alloc_tile_pool(name="psA", bufs=2, space="PSUM")
    psB = tc.alloc_tile_pool(name="psB", bufs=4, space="PSUM")

    vp_view = vp_sb.rearrange("p st (h c) -> p st h c", c=HD + 1)
    nc.gpsimd.memset(vp_view[:, :, :, HD:HD + 1], 1.0)

    exp_warm = top.tile([1, 8], F32, name="exp_warm")
    nc.scalar.activation(exp_warm, ones[0:1, 0:8], AF.Exp, scale=1.0)

    # ---- phase 1: x^T transposes interleaved with V matmuls, per seq tile --
    def emit_v(st):
        for nb, (noff, nw) in enumerate(((0, 512), (512, 256))):
            ps = psB.tile([128, 512], F32, tag="o", name=f"v_{st}_{nb}")
            seg = ps[:, 0:nw]
            for kt_i in range(DT):
                nc.tensor.matmul(
                    seg,
                    xt_sb[:, kt_i, st * 128:(st + 1) * 128],
                    wv_sb[:, kt_i, noff:noff + nw],
                    start=(kt_i == 0),
                    stop=(kt_i == DT - 1 and not with_bias),
                )
            if with_bias:
                nc.tensor.matmul(
                    seg,
                    ones[0:1, 0:128],
                    bq_row[0:1, 2 * D + noff:2 * D + noff + nw],
                    start=False, stop=True,
                )
            h0, hn = noff // HD, nw // HD
            nc.vector.tensor_copy(
                vp_view[:, st, h0:h0 + hn, 0:HD],
                seg.rearrange("p (h c) -> p h c", c=HD),
            )

    for st in range(NT):
        x_t = xs_pool.tile([128, D], F32, tag="xrow", name=f"x_{st}")
        nc.sync.dma_start(x_t, x_d.ap()[st * 128:(st + 1) * 128, :])
        for dt_i in range(DT):
            pt = psA.tile([128, 128], F32, tag="s", name=f"pt_{st}_{dt_i}")
            nc.tensor.transpose(pt, x_t[:, dt_i * 128:(dt_i + 1) * 128], identity)
            nc.scalar.copy(xt_sb[:, dt_i, st * 128:(st + 1) * 128], pt)
        if st >= 2:
            emit_v(st - 2)  # V lags two tiles: overlap + wv DMA arrival time
    emit_v(NT - 2)
    emit_v(NT - 1)

    xs_pool.release()

    # ---- phase 2: Q^T / K^T ----
    def emit_qk(mt):
        for which, dst in ((0, qt_sb), (1, kt_sb)):
            ps = psA.tile([128, N], F32, tag="s", name=f"qk_{which}_{mt}")
            for qb in range(2):
                seg = ps[:, qb * 512:(qb + 1) * 512]
                for kt_i in range(DT):
                    nc.tensor.matmul(
                        seg,
                        wqk_sb[:, kt_i, which * D + mt * 128:
                               which * D + (mt + 1) * 128],
                        xt_sb[:, kt_i, qb * 512:(qb + 1) * 512],
                        start=(kt_i == 0),
                        stop=(kt_i == DT - 1 and not with_bias),
                    )
                if with_bias:
                    nc.tensor.matmul(
                        seg,
                        bq_row[0:1, which * D + mt * 128:
                               which * D + (mt + 1) * 128],
                        ones[0:1, 0:512],
                        start=False, stop=True,
                    )
            nc.scalar.copy(dst[:, mt, :], ps)

    for mt in range(DT):
        emit_qk(mt)
    wqk_pool.release()
    wv_pool0.release()
    xt_pool.release()

    # ---- phase 3: attention, head pairs ----
    late = tc.alloc_tile_pool(name="late", bufs=1)
    wp_sb = late.tile([128, DT, D], F32R, name="wp_sb")
    nc.sync.dma_start(
        wp_sb, wp_d.ap().rearrange("(ko p) n -> p ko n", p=128).bitcast(F32R))
    # Per-pair softmax-denominator and reciprocal tiles (partitions 0-1).
    spair = [late.tile([2, N], F32, name=f"spair_{p}") for p in range(H // 2)]
    rpair = [late.tile([2, N], F32, name=f"rpair_{p}") for p in range(H // 2)]
    pexp_pool = tc.alloc_tile_pool(name="pexp", bufs=5)
    stage_pool = tc.alloc_tile_pool(name="stage", bufs=4)
    flat_pool = tc.alloc_tile_pool(name="flat", bufs=3)
    flats = {}

    def emit_norm(pr):
        # Broadcast 1/s to the 64 head rows via K=1 f32 matmul, multiply
        # into O^T. Runs two pairs later so psB slots and the chain are free.
        for qb in range(2):
            r_ps = psB.tile([128, 512], F32, tag="o", name=f"r_{pr}_{qb}")
            for hh in range(2):
                po = 64 * hh
                src_row = rpair[pr][0:1] if hh == 0 else flats[pr][0:1]
                nc.tensor.matmul(
                    r_ps[po:po + 64, :],
                    ones[0:1, 0:HD],
                    src_row[:, qb * 512:(qb + 1) * 512],
                    start=True, stop=True,
                )
            for hh in range(2):
                po = 64 * hh
                dst = ot_sb[po:po + 64, pr, qb * 512:(qb + 1) * 512]
                nc.vector.tensor_mul(out=dst, in0=dst, in1=r_ps[po:po + 64, :])

    for pr in range(H // 2):  # heads (2*pr, 2*pr+1); Q/K tile mt = pr
        if pr >= 2:
            emit_norm(pr - 2)
        o_ps = {}
        for hh in range(2):
            for qb in range(2):
                o_ps[(hh, qb)] = psB.tile(
                    [65, 512], F32, tag="o", name=f"o_{pr}_{hh}_{qb}")

        for kt_i in range(NT):
            pexp = {}
            for hh in range(2):
                po = 64 * hh
                s_ps = psA.tile([128, N], F32, tag="s",
                                name=f"s_{pr}_{kt_i}_{hh}")
                for qb in range(2):
                    nc.tensor.matmul(
                        s_ps[:, qb * 512:(qb + 1) * 512],
                        kt_sb[po:po + 64, pr, kt_i * 128:(kt_i + 1) * 128],
                        qt_sb[po:po + 64, pr, qb * 512:(qb + 1) * 512],
                        start=True, stop=True,
                    )
                pe = pexp_pool.tile([128, N], BF16, tag="pexp",
                                    name=f"pe_{pr}_{kt_i}_{hh}")
                nc.scalar.activation(pe, s_ps, AF.Exp, scale=float(SCALE))
                pexp[hh] = pe

            for hh in range(2):
                h = 2 * pr + hh
                for qb in range(2):
                    nc.tensor.matmul(
                        o_ps[(hh, qb)],
                        vp_sb[:, kt_i, h * (HD + 1):(h + 1) * (HD + 1)],
                        pexp[hh][:, qb * 512:(qb + 1) * 512],
                        start=(kt_i == 0),
                        stop=(kt_i == NT - 1),
                        skip_group_check=True,
                    )

        for hh in range(2):
            h = 2 * pr + hh
            po = 64 * hh
            for qb in range(2):
                stg = stage_pool.tile([65, 512], F32, tag="stage",
                                      name=f"stg_{h}_{qb}")
                nc.vector.tensor_copy(stg, o_ps[(hh, qb)])
                nc.sync.dma_start(
                    ot_sb[po:po + 64, pr, qb * 512:(qb + 1) * 512],
                    stg[0:HD, :].bitcast(F32R))
                nc.sync.dma_start(
                    spair[pr][hh:hh + 1, qb * 512:(qb + 1) * 512],
                    stg[HD:HD + 1, :])

        nc.vector.reciprocal_approx_fast(out=rpair[pr], in_=spair[pr])
        fl = flat_pool.tile([1, N], F32, tag="flat", name=f"fl_{pr}")
        nc.sync.dma_start(fl, rpair[pr][1:2, :])
        flats[pr] = fl

    emit_norm(H // 2 - 2)
    pr_last = H // 2 - 1
    r_ps = psA.tile([128, N], F32, tag="s", name="rA_last")
    for qb in range(2):
        for hh in range(2):
            po = 64 * hh
            src_row = rpair[pr_last][0:1] if hh == 0 else flats[pr_last][0:1]
            nc.tensor.matmul(
                r_ps[po:po + 64, qb * 512:(qb + 1) * 512],
                ones[0:1, 0:HD],
                src_row[:, qb * 512:(qb + 1) * 512],
                start=True, stop=True,
            )
    for qb in range(2):
        for hh in range(2):
            po = 64 * hh
            dst = ot_sb[po:po + 64, pr_last, qb * 512:(qb + 1) * 512]
            nc.vector.tensor_mul(
                out=dst, in0=dst,
                in1=r_ps[po:po + 64, qb * 512:(qb + 1) * 512])
    flat_pool.release()
    stage_pool.release()
    pexp_pool.release()

    # ---- phase 4: out = O @ w_proj (+ b_proj) ----
    fout_pool = tc.alloc_tile_pool(name="fout", bufs=3)
    for st in range(NT):
        f_ps = psA.tile([128, D], F32, tag="s", name=f"f_{st}")
        for noff, nw in ((0, 512), (512, 256)):
            seg = f_ps[:, noff:noff + nw]
            for kt_i in range(DT):
                nc.tensor.matmul(
                    seg,
                    ot_sb[:, kt_i, st * 128:(st + 1) * 128],
                    wp_sb[:, kt_i, noff:noff + nw],
                    start=(kt_i == 0),
                    stop=(kt_i == DT - 1 and not with_bias),
                )
            if with_bias:
                nc.tensor.matmul(
                    seg,
                    ones[0:1, 0:128],
                    bp_row[0:1, noff:noff + nw],
                    start=False, stop=True,
                )
        fo = fout_pool.tile([128, D], F32, tag="fout", name=f"fo_{st}")
        if st % 2 == 0:
            nc.vector.tensor_copy(fo, f_ps)
        else:
            nc.scalar.copy(fo, f_ps)
        nc.sync.dma_start(out_d.ap()[st * 128:(st + 1) * 128, :], fo)

    fout_pool.release()
    late.release()
    psB.release()
    psA.release()
    top.release()


_module_cache: dict = {}


def get_module(with_bias: bool) -> bass.Bass:
    if with_bias not in _module_cache:
        _module_cache[with_bias] = build_module(with_bias)
    return _module_cache[with_bias]


def kernel(x, w_qkv, b_qkv, w_proj, b_proj):
    from concourse.bass_utils import run_bass_kernel_spmd

    x = np.ascontiguousarray(np.asarray(x, dtype=np.float32))
    w_qkv = np.ascontiguousarray(np.asarray(w_qkv, dtype=np.float32))
    b_qkv = np.ascontiguousarray(np.asarray(b_qkv, dtype=np.float32)).reshape(1, 3 * D)
    w_proj = np.ascontiguousarray(np.asarray(w_proj, dtype=np.float32))
    b_proj = np.ascontiguousarray(np.asarray(b_proj, dtype=np.float32)).reshape(1, D)

    B = x.shape[0]
    assert x.shape == (B, N, D) and B == 8, x.shape

    with_bias = bool(np.any(b_qkv) or np.any(b_proj))
    nc = get_module(with_bias)

    in_maps = [
        {
            "x": np.ascontiguousarray(x[b]),
            "w_qkv": w_qkv,
            "b_qkv": b_qkv,
            "w_proj": w_proj,
            "b_proj": b_proj,
        }
        for b in range(B)
    ]
    res = run_bass_kernel_spmd(nc, in_maps, core_ids=list(range(B)))
    kernel.last_results = res
    return np.stack([res.results[b]["out"] for b in range(B)], axis=0)

